# revision 18
# baseline (speedup 1.0000x reference)
"""Trainium2 Bass kernel for a 24-layer GPT-2-medium-style LM (tied lm_head + CE loss).

Sharding: DP-2 over batch x TP-4 (megatron) within each half of the 8 cores.
  - group g = cores [4g .. 4g+3] handles batch sample g (T=1024 tokens).
  - within a group, rank r shards: attention heads (4/core), FFN inner dim
    (1024/core), and the tied vocab lm_head (12800 padded cols/core).
  - 2 AllReduces per layer (attn proj partial, fc proj partial) over each
    group of 4 cores.

On-chip layout: activations kept feature-major x_fm[d_part, d_sub, token] so
every linear contracts d on partitions.  LayerNorm mean/sumsq are computed
with ones-vector matmuls (partition reductions) in float32r; per-token scale/
shift vectors are broadcast across partitions with rank-1 matmuls.  Softmax
skips max-subtraction (|scores| < 4) and folds the normalizer into the AV
matmul via an appended ones-column on V; the reciprocal is applied on PSUM
eviction.  LN gains/biases are folded into the adjacent weights on the host.

kernel(**inputs) takes the FULL inputs (as produced by setup_inputs) and
returns (logits [2,1024,50257] fp32, loss fp32) like the reference.
"""

import numpy as np
import ml_dtypes

# model dims (hardcoded for this problem)
V, D, H, HD, L, FF = 50257, 1024, 16, 64, 24, 4096
B, T = 2, 1024
EPS = 1e-5

P = 128
DS = D // P            # 8 feature subtiles
N_CORES = 8
TPG = 4                # tensor-parallel group size
NH = H // TPG          # 4 local heads
QC = NH * HD           # 256 local q/k/v cols
MS = QC // P           # 2 subtiles of q/k/y
FFL = FF // TPG        # 1024 local ffn cols
FS = FFL // P          # 8
TT = T // P            # 8 token tiles
CH = 512               # token chunk (matmul moving free dim)
NCH = T // CH          # 2
VLOC = 12800           # padded vocab shard per core (4*12800 = 51200 >= V)
VT = VLOC // CH        # 25 vocab tiles
RG = [[0, 1, 2, 3], [4, 5, 6, 7]]

_CACHE = {}


def _bf16(x):
    return np.ascontiguousarray(np.asarray(x, np.float32).astype(ml_dtypes.bfloat16))


def _f32(x):
    return np.ascontiguousarray(np.asarray(x, np.float32))


def _fm(arr):
    """[D, M] -> lhsT/rhs SBUF layout [P, D//P, M] with d = ds*P + p."""
    d, m = arr.shape
    return np.ascontiguousarray(arr.reshape(d // P, P, m).transpose(1, 0, 2))


def _percol(vec):
    """[n*P] -> per-partition bias layout [P, n] with col index = ns*P + p."""
    n = vec.shape[0] // P
    return np.ascontiguousarray(vec.reshape(n, P).T)


def _build_program(use_vb):
    import concourse.bass as bass  # noqa: F401
    import concourse.tile as tile
    from concourse import bacc, mybir
    from contextlib import ExitStack

    dt = mybir.dt
    Alu = mybir.AluOpType
    Act = mybir.ActivationFunctionType

    nc = bacc.Bacc(None, target_bir_lowering=False)

    # ---- per-core DRAM I/O ----
    x0_d = nc.dram_tensor("x0", [P, DS, T], dt.float32, kind="ExternalInput")
    wq_d = nc.dram_tensor("wq", [L, P, DS, QC], dt.bfloat16, kind="ExternalInput")
    wk_d = nc.dram_tensor("wk", [L, P, DS, QC], dt.bfloat16, kind="ExternalInput")
    wv_d = nc.dram_tensor("wv", [L, P, DS, QC], dt.bfloat16, kind="ExternalInput")
    bq_d = nc.dram_tensor("bq", [L, P, MS], dt.float32, kind="ExternalInput")
    bk_d = nc.dram_tensor("bk", [L, P, MS], dt.float32, kind="ExternalInput")
    bv_d = nc.dram_tensor("bv", [L, 1, QC], dt.bfloat16, kind="ExternalInput")
    wp_d = nc.dram_tensor("wp", [L, P, MS, D], dt.bfloat16, kind="ExternalInput")
    pb_d = nc.dram_tensor("pb", [L, P, DS], dt.float32, kind="ExternalInput")
    wfc_d = nc.dram_tensor("wfc", [L, P, DS, FFL], dt.bfloat16, kind="ExternalInput")
    fcb_d = nc.dram_tensor("fcb", [L, P, FS], dt.float32, kind="ExternalInput")
    wfp_d = nc.dram_tensor("wfp", [L, P, FS, D], dt.bfloat16, kind="ExternalInput")
    fpb_d = nc.dram_tensor("fpb", [L, P, DS], dt.float32, kind="ExternalInput")
    wvoc_d = nc.dram_tensor("wvoc", [P, DS, VLOC], dt.bfloat16, kind="ExternalInput")
    vb_d = nc.dram_tensor("vb", [1, VLOC], dt.bfloat16, kind="ExternalInput")
    mask_d = nc.dram_tensor("mask", [P, P], dt.bfloat16, kind="ExternalInput")

    logits_d = nc.dram_tensor("logits", [TT, P, VLOC], dt.float32, kind="ExternalOutput")
    sumexp_d = nc.dram_tensor("sumexp", [P, TT], dt.float32, kind="ExternalOutput")

    with tile.TileContext(nc) as tc, ExitStack() as ctx:
        const = ctx.enter_context(tc.tile_pool(name="const", bufs=1))
        pers = ctx.enter_context(tc.tile_pool(name="pers", bufs=1))
        wpool = ctx.enter_context(tc.tile_pool(name="w", bufs=1))
        wvpool = ctx.enter_context(tc.tile_pool(name="wv2", bufs=2))
        work = ctx.enter_context(tc.tile_pool(name="work", bufs=3))
        work2 = ctx.enter_context(tc.tile_pool(name="work2", bufs=2))
        tiny1 = ctx.enter_context(tc.tile_pool(name="tiny1", bufs=1))
        tiny2 = ctx.enter_context(tc.tile_pool(name="tiny2", bufs=2))
        wstream = ctx.enter_context(tc.tile_pool(name="wstream", bufs=3))
        bcp = ctx.enter_context(tc.tile_pool(name="bcp", bufs=2))
        zp = ctx.enter_context(tc.tile_pool(name="zp", bufs=2))
        ps = ctx.enter_context(tc.tile_pool(name="ps", bufs=4, space="PSUM"))
        pstat = ctx.enter_context(tc.tile_pool(name="pstat", bufs=2, space="PSUM"))
        pbc = ctx.enter_context(tc.tile_pool(name="pbc", bufs=2, space="PSUM"))
        dram = ctx.enter_context(tc.tile_pool(name="dram", bufs=2, space="DRAM"))

        # constants
        ones_col_f = const.tile([P, 1], dt.float32)
        nc.vector.memset(ones_col_f[:], 1.0)
        ones_col_r = const.tile([P, 1], dt.float32r)
        nc.vector.tensor_copy(ones_col_r[:], ones_col_f[:])
        ones_col_b = const.tile([P, 1], dt.bfloat16)
        nc.vector.memset(ones_col_b[:], 1.0)
        ones_row_f = const.tile([1, P], dt.float32)
        nc.vector.memset(ones_row_f[:], 1.0)
        ones_row_r = const.tile([1, P], dt.float32r)
        nc.vector.tensor_copy(ones_row_r[:], ones_row_f[:])
        ones_row_b = const.tile([1, P], dt.bfloat16)
        nc.vector.memset(ones_row_b[:], 1.0)
        mask_sb = const.tile([P, P], dt.bfloat16)
        nc.sync.dma_start(mask_sb[:], mask_d[:])
        eps_sb = const.tile([1, 1], dt.float32)
        nc.vector.memset(eps_sb[:], EPS)

        # persistent activations
        x_fm = pers.tile([P, DS, T], dt.float32)
        h_sb = pers.tile([P, DS, T], dt.bfloat16)
        q_fm = pers.tile([P, MS, T], dt.bfloat16)
        k_fm = pers.tile([P, MS, T], dt.bfloat16)
        v_aug = pers.tile([P, TT, NH, HD + 1], dt.bfloat16)
        y_fm = pers.tile([P, MS, T], dt.bfloat16)
        ffn_sb = pers.tile([P, FS, T], dt.bfloat16)
        accs = pers.tile([P, TT, VT], dt.float32)
        sumexp_sb = pers.tile([P, TT], dt.float32)

        nc.vector.memset(v_aug[:, :, :, HD:HD + 1], 1.0)
        nc.sync.dma_start(x_fm[:], x0_d[:])

        def layernorm():
            """h_sb = LN(x_fm) without gain/bias (folded into weights), bf16."""
            for c in range(NCH):
                tcs = slice(c * CH, (c + 1) * CH)
                p_sum = pstat.tile([1, CH], dt.float32, tag="pstat")
                for d_i in range(DS):
                    xr = work.tile([P, CH], dt.float32r, tag="xr")
                    nc.vector.tensor_copy(xr[:], x_fm[:, d_i, tcs])
                    nc.tensor.matmul(
                        p_sum[:], ones_col_r[:], xr[:],
                        start=d_i == 0, stop=d_i == DS - 1,
                    )
                mu_t = tiny1.tile([1, CH], dt.float32, tag="mu")
                nc.vector.tensor_scalar(
                    mu_t[:], p_sum[:], 1.0 / D, None, Alu.mult)
                p_sq = pstat.tile([1, CH], dt.float32, tag="pstat")
                for d_i in range(DS):
                    xsq = work.tile([P, CH], dt.bfloat16, tag="xsq")
                    nc.scalar.activation(xsq[:], x_fm[:, d_i, tcs], Act.Square)
                    nc.tensor.matmul(
                        p_sq[:], ones_col_b[:], xsq[:],
                        start=d_i == 0, stop=d_i == DS - 1,
                    )
                musq = tiny1.tile([1, CH], dt.float32, tag="musq")
                nc.vector.tensor_tensor(
                    musq[:], mu_t[:], mu_t[:], Alu.mult)
                var = tiny1.tile([1, CH], dt.float32, tag="var")
                nc.vector.scalar_tensor_tensor(
                    var[:], p_sq[:], 1.0 / D, musq[:], Alu.mult, Alu.subtract)
                std = tiny1.tile([1, CH], dt.float32, tag="std")
                nc.scalar.activation(std[:], var[:], Act.Sqrt, bias=eps_sb[0:1, 0:1])
                inv_t = tiny1.tile([1, CH], dt.float32r, tag="inv")
                with nc.allow_low_precision(reason="f32r rounding for broadcast matmul"):
                    nc.vector.reciprocal(inv_t[:], std[:])
                csc_t = tiny1.tile([1, CH], dt.float32r, tag="csc")
                nc.vector.scalar_tensor_tensor(
                    csc_t[:], mu_t[:], -1.0, inv_t[:],
                    Alu.mult, Alu.mult)
                # broadcast a = inv, c = -mu*inv across partitions
                p_a = pbc.tile([P, CH], dt.float32, tag="pbc")
                nc.tensor.matmul(
                    p_a[:], ones_row_r[:], inv_t[:],
                    start=True, stop=True)
                a_bc = bcp.tile([P, CH], dt.float32, tag="abc")
                nc.vector.tensor_copy(a_bc[:], p_a[:])
                p_c = pbc.tile([P, CH], dt.float32, tag="pbc")
                nc.tensor.matmul(
                    p_c[:], ones_row_r[:], csc_t[:],
                    start=True, stop=True)
                c_bc = bcp.tile([P, CH], dt.float32, tag="cbc")
                nc.vector.tensor_copy(c_bc[:], p_c[:])
                for d_i in range(DS):
                    tmp = work2.tile([P, CH], dt.float32, tag="lntmp")
                    nc.vector.tensor_tensor(
                        tmp[:], x_fm[:, d_i, tcs], a_bc[:], Alu.mult)
                    nc.vector.tensor_tensor(
                        h_sb[:, d_i, tcs], tmp[:], c_bc[:], Alu.add)

        def residual_add(bout, bias_sb):
            """x_fm += AR_result + bias (bias per out-col, [P, DS])."""
            RC = 256
            for c4 in range(T // RC):
                cs = slice(c4 * RC, (c4 + 1) * RC)
                zs = zp.tile([P, DS, RC], dt.float32, tag="zr")
                nc.sync.dma_start(zs[:], bout[:, :, cs])
                for n_i in range(DS):
                    nc.vector.scalar_tensor_tensor(
                        x_fm[:, n_i, cs], zs[:, n_i, :], bias_sb[:, n_i:n_i + 1],
                        x_fm[:, n_i, cs], Alu.add, Alu.add)

        for li in range(L):
            # ---- load layer weights ----
            wq_sb = wpool.tile([P, DS, QC], dt.bfloat16, tag="wq")
            nc.sync.dma_start(wq_sb[:], wq_d[li])
            wk_sb = wpool.tile([P, DS, QC], dt.bfloat16, tag="wk")
            nc.sync.dma_start(wk_sb[:], wk_d[li])
            wv_sb = wpool.tile([P, DS, QC], dt.bfloat16, tag="wv")
            nc.sync.dma_start(wv_sb[:], wv_d[li])
            bq_sb = wpool.tile([P, MS], dt.float32, tag="bq")
            nc.sync.dma_start(bq_sb[:], bq_d[li])
            bk_sb = wpool.tile([P, MS], dt.float32, tag="bk")
            nc.sync.dma_start(bk_sb[:], bk_d[li])
            bv_sb = wpool.tile([1, QC], dt.bfloat16, tag="bv")
            nc.sync.dma_start(bv_sb[:], bv_d[li])
            wp_sb = wpool.tile([P, MS, D], dt.bfloat16, tag="wp")
            nc.sync.dma_start(wp_sb[:], wp_d[li])
            pb_sb = wpool.tile([P, DS], dt.float32, tag="pb")
            nc.sync.dma_start(pb_sb[:], pb_d[li])
            fcb_sb = wpool.tile([P, FS], dt.float32, tag="fcb")
            nc.sync.dma_start(fcb_sb[:], fcb_d[li])
            fpb_sb = wpool.tile([P, DS], dt.float32, tag="fpb")
            nc.sync.dma_start(fpb_sb[:], fpb_d[li])

            # ---- ln1 + qkv ----
            layernorm()
            for m_i in range(MS):
                mslice = slice(m_i * P, (m_i + 1) * P)
                for c in range(NCH):
                    tcs = slice(c * CH, (c + 1) * CH)
                    pq = ps.tile([P, CH], dt.float32, tag="ps")
                    for d_i in range(DS):
                        nc.tensor.matmul(
                            pq[:], wq_sb[:, d_i, mslice], h_sb[:, d_i, tcs],
                            start=d_i == 0, stop=d_i == DS - 1)
                    nc.vector.tensor_scalar(
                        q_fm[:, m_i, tcs], pq[:], bq_sb[:, m_i:m_i + 1], None,
                        Alu.add)
                    pk = ps.tile([P, CH], dt.float32, tag="ps")
                    for d_i in range(DS):
                        nc.tensor.matmul(
                            pk[:], wk_sb[:, d_i, mslice], h_sb[:, d_i, tcs],
                            start=d_i == 0, stop=d_i == DS - 1)
                    nc.vector.tensor_scalar(
                        k_fm[:, m_i, tcs], pk[:], bk_sb[:, m_i:m_i + 1], None,
                        Alu.add)
            use_bv = True  # bias-in-psum matmul for token-major V
            for t_i in range(TT):
                tts = slice(t_i * P, (t_i + 1) * P)
                pv = ps.tile([P, QC], dt.float32, tag="ps")
                for d_i in range(DS):
                    nc.tensor.matmul(
                        pv[:], h_sb[:, d_i, tts], wv_sb[:, d_i, :],
                        start=d_i == 0, stop=(not use_bv) and d_i == DS - 1)
                if use_bv:
                    nc.tensor.matmul(
                        pv[:], ones_row_b[:], bv_sb[0:1, :],
                        start=False, stop=True)
                nc.vector.tensor_copy(
                    v_aug[:, t_i, :, 0:HD],
                    pv[:].rearrange("p (h d) -> p h d", h=NH))

            # ---- attention ----
            for hh in range(NH):
                po = (hh % 2) * HD
                m_i = hh // 2
                for qc_ in range(NCH):
                    tq0 = qc_ * CH
                    ktiles = list(range(min(TT, qc_ * 4 + 4)))
                    py = ps.tile([HD, CH], dt.float32, tag="ps")
                    pn = pstat.tile([1, CH], dt.float32, tag="pstat")
                    for idx, tkt in enumerate(ktiles):
                        pscr = ps.tile([P, CH], dt.float32, tag="ps")
                        nc.tensor.matmul(
                            pscr[:],
                            k_fm[po:po + HD, m_i, tkt * P:(tkt + 1) * P],
                            q_fm[po:po + HD, m_i, tq0:tq0 + CH],
                            start=True, stop=True)
                        att = work.tile([P, CH], dt.bfloat16, tag="att")
                        nc.scalar.activation(att[:], pscr[:], Act.Exp)
                        o = tkt - qc_ * 4
                        if o >= 0:
                            if o > 0:
                                nc.vector.memset(att[:, 0:o * P], 0.0)
                            nc.vector.tensor_tensor(
                                att[:, o * P:(o + 1) * P],
                                att[:, o * P:(o + 1) * P],
                                mask_sb[:], Alu.mult)
                        nc.tensor.matmul(
                            py[:], v_aug[:, tkt, hh, 0:HD], att[:],
                            start=idx == 0, stop=idx == len(ktiles) - 1)
                        nc.tensor.matmul(
                            pn[:], v_aug[:, tkt, hh, HD:HD + 1], att[:],
                            start=idx == 0, stop=idx == len(ktiles) - 1)
                    rcp = tiny2.tile([1, CH], dt.float32r, tag="rcp")
                    with nc.allow_low_precision(reason="f32r rounding for broadcast matmul"):
                        nc.vector.reciprocal(rcp[:], pn[:])
                    p_r = pbc.tile([HD, CH], dt.float32, tag="pbc")
                    nc.tensor.matmul(
                        p_r[:], ones_row_r[0:1, 0:HD],
                        rcp[:], start=True, stop=True)
                    rbc = bcp.tile([HD, CH], dt.float32, tag="rbc")
                    nc.vector.tensor_copy(rbc[:], p_r[:])
                    if po == 0:
                        nc.vector.tensor_tensor(
                            y_fm[0:HD, m_i, tq0:tq0 + CH], py[0:HD, :], rbc[:],
                            Alu.mult)
                    else:
                        yst = work.tile([HD, CH], dt.bfloat16, tag="yst")
                        nc.vector.tensor_tensor(
                            yst[:], py[0:HD, :], rbc[:], Alu.mult)
                        nc.sync.dma_start(y_fm[po:po + HD, m_i, tq0:tq0 + CH], yst[:])

            # ---- attn proj + AllReduce + residual ----
            bin_a = dram.tile([P, DS, T], dt.float32, tag="arin")
            bout_a = dram.tile([P, DS, T], dt.float32, tag="arout")
            for n_i in range(DS):
                for c in range(NCH):
                    tcs = slice(c * CH, (c + 1) * CH)
                    pz = ps.tile([P, CH], dt.float32, tag="ps")
                    for k_i in range(MS):
                        nc.tensor.matmul(
                            pz[:], wp_sb[:, k_i, n_i * P:(n_i + 1) * P],
                            y_fm[:, k_i, tcs],
                            start=k_i == 0, stop=k_i == MS - 1)
                    zev = work2.tile([P, CH], dt.float32, tag="zev")
                    nc.vector.tensor_copy(zev[:], pz[:])
                    nc.sync.dma_start(bin_a[:, n_i, tcs], zev[:])
            nc.gpsimd.collective_compute(
                "AllReduce", Alu.add, replica_groups=RG,
                ins=[bin_a.opt()], outs=[bout_a.opt()])
            residual_add(bout_a, pb_sb)

            # ---- ln2 + ffn ----
            layernorm()
            for f_i in range(FS):
                fslice = slice(f_i * P, (f_i + 1) * P)
                wfct = wstream.tile([P, DS, P], dt.bfloat16, tag="wfc")
                nc.sync.dma_start(wfct[:], wfc_d[li][:, :, fslice])
                for c in range(NCH):
                    tcs = slice(c * CH, (c + 1) * CH)
                    pf = ps.tile([P, CH], dt.float32, tag="ps")
                    for d_i in range(DS):
                        nc.tensor.matmul(
                            pf[:], wfct[:, d_i, :], h_sb[:, d_i, tcs],
                            start=d_i == 0, stop=d_i == DS - 1)
                    nc.scalar.activation(
                        ffn_sb[:, f_i, tcs], pf[:], Act.Gelu,
                        bias=fcb_sb[:, f_i:f_i + 1])
            bin_b = dram.tile([P, DS, T], dt.float32, tag="arin2")
            bout_b = dram.tile([P, DS, T], dt.float32, tag="arout2")
            for n_i in range(DS):
                wfpt = wstream.tile([P, FS, P], dt.bfloat16, tag="wfp")
                nc.sync.dma_start(wfpt[:], wfp_d[li][:, :, n_i * P:(n_i + 1) * P])
                for c in range(NCH):
                    tcs = slice(c * CH, (c + 1) * CH)
                    pz = ps.tile([P, CH], dt.float32, tag="ps")
                    for f_i in range(FS):
                        nc.tensor.matmul(
                            pz[:], wfpt[:, f_i, :],
                            ffn_sb[:, f_i, tcs],
                            start=f_i == 0, stop=f_i == FS - 1)
                    zev = work2.tile([P, CH], dt.float32, tag="zev")
                    nc.vector.tensor_copy(zev[:], pz[:])
                    nc.sync.dma_start(bin_b[:, n_i, tcs], zev[:])
            nc.gpsimd.collective_compute(
                "AllReduce", Alu.add, replica_groups=RG,
                ins=[bin_b.opt()], outs=[bout_b.opt()])
            residual_add(bout_b, fpb_sb)

        # ---- final LN + lm_head + sumexp stats ----
        layernorm()
        vb_sb = None
        if use_vb:
            vb_sb = const.tile([1, VLOC], dt.bfloat16)
            nc.sync.dma_start(vb_sb[:], vb_d[:])
        for vc in range(VT):
            vcs = slice(vc * CH, (vc + 1) * CH)
            wvc = wvpool.tile([P, DS, CH], dt.bfloat16, tag="wvoc")
            nc.sync.dma_start(wvc[:], wvoc_d[:, :, vcs])
            for t_i in range(TT):
                tts = slice(t_i * P, (t_i + 1) * P)
                pl = ps.tile([P, CH], dt.float32, tag="ps")
                for d_i in range(DS):
                    nc.tensor.matmul(
                        pl[:], h_sb[:, d_i, tts], wvc[:, d_i, :],
                        start=d_i == 0, stop=(not use_vb) and d_i == DS - 1)
                if use_vb:
                    nc.tensor.matmul(
                        pl[:], ones_row_b[:], vb_sb[0:1, vcs],
                        start=False, stop=True)
                lgev = work2.tile([P, CH], dt.float32, tag="lgev")
                nc.vector.tensor_copy(lgev[:], pl[:])
                nc.sync.dma_start(logits_d[t_i, :, vcs], lgev[:])
                esc = work2.tile([P, CH], dt.float32, tag="esc")
                nc.scalar.activation(
                    esc[:], lgev[:], Act.Exp, accum_out=accs[:, t_i, vc:vc + 1])
        for t_i in range(TT):
            nc.vector.tensor_reduce(
                sumexp_sb[:, t_i:t_i + 1], accs[:, t_i, :],
                mybir.AxisListType.X, Alu.add)
        nc.sync.dma_start(sumexp_d[:], sumexp_sb[:])

    nc.compile()
    return nc


def _prep_in_maps(input_ids, labels, params):
    p = {k: _f32(v) for k, v in params.items()}
    ids = np.asarray(input_ids)
    sc = np.float32(1.0 / np.sqrt(HD))

    vb = p['wte'] @ p['lnf_b']          # [V] logit bias from folded lnf_b
    use_vb = bool(np.any(vb != 0.0))

    # per-layer folded weights (shared across cores; sliced per rank)
    wq_l, wk_l, wv_l = [], [], []
    bq_l, bk_l, bv_l = [], [], []
    wp_l, pb_l, wfc_l, fcb_l, wfp_l, fpb_l = [], [], [], [], [], []
    for l in range(L):
        w_eff = p['ln1_g'][l][:, None] * p['attn_w'][l]         # [D, 3D]
        b_eff = p['attn_b'][l] + p['ln1_b'][l] @ p['attn_w'][l]  # [3D]
        w_eff = w_eff.copy()
        b_eff = b_eff.copy()
        w_eff[:, :D] *= sc
        b_eff[:D] *= sc
        wq_l.append(w_eff[:, 0:D])
        wk_l.append(w_eff[:, D:2 * D])
        wv_l.append(w_eff[:, 2 * D:3 * D])
        bq_l.append(b_eff[0:D])
        bk_l.append(b_eff[D:2 * D])
        bv_l.append(b_eff[2 * D:3 * D])
        wp_l.append(p['attn_proj_w'][l])
        pb_l.append(p['attn_proj_b'][l])
        wfc_l.append(p['ln2_g'][l][:, None] * p['fc_w'][l])
        fcb_l.append(p['fc_b'][l] + p['ln2_b'][l] @ p['fc_w'][l])
        wfp_l.append(p['fc_proj_w'][l])
        fpb_l.append(p['fc_proj_b'][l])

    wteT_eff = (p['lnf_g'][:, None] * p['wte'].T)                # [D, V]
    mask_np = _bf16(np.triu(np.ones((P, P), np.float32)))

    in_maps = []
    meta = []
    for core in range(N_CORES):
        g, r = divmod(core, TPG)
        qs = slice(r * QC, (r + 1) * QC)
        fsl = slice(r * FFL, (r + 1) * FFL)
        v0 = r * VLOC
        vw = max(0, min(VLOC, V - v0))       # real vocab width of this shard
        wvoc = np.zeros((D, VLOC), np.float32)
        wvoc[:, :vw] = wteT_eff[:, v0:v0 + vw]
        vb_core = np.full((VLOC,), -1e4 if use_vb else 0.0, np.float32)
        vb_core[:vw] = vb[v0:v0 + vw]

        x = p['wte'][ids[g]] + p['wpe'][:T]          # [T, D]
        m = {
            'x0': _f32(_fm(x.T)),
            'wq': _bf16(np.stack([_fm(wq_l[l][:, qs]) for l in range(L)])),
            'wk': _bf16(np.stack([_fm(wk_l[l][:, qs]) for l in range(L)])),
            'wv': _bf16(np.stack([_fm(wv_l[l][:, qs]) for l in range(L)])),
            'bq': _f32(np.stack([_percol(bq_l[l][qs]) for l in range(L)])),
            'bk': _f32(np.stack([_percol(bk_l[l][qs]) for l in range(L)])),
            'bv': _bf16(np.stack([bv_l[l][qs][None, :] for l in range(L)])),
            'wp': _bf16(np.stack([_fm(wp_l[l][qs, :]) for l in range(L)])),
            'pb': _f32(np.stack([_percol(pb_l[l]) for l in range(L)])),
            'wfc': _bf16(np.stack([_fm(wfc_l[l][:, fsl]) for l in range(L)])),
            'fcb': _f32(np.stack([_percol(fcb_l[l][fsl]) for l in range(L)])),
            'wfp': _bf16(np.stack([_fm(wfp_l[l][fsl, :]) for l in range(L)])),
            'fpb': _f32(np.stack([_percol(fpb_l[l]) for l in range(L)])),
            'wvoc': _bf16(_fm(wvoc)),
            'vb': _bf16(vb_core[None, :]),
            'mask': mask_np,
        }
        in_maps.append(m)
        meta.append({'g': g, 'r': r, 'v0': v0, 'vw': vw,
                     'padcount': 0 if use_vb else (VLOC - vw)})
    return in_maps, meta, use_vb


def run_on_device(input_ids, labels, params, trace=False):
    """Returns (logits [B,T,V] f32, loss f32, exec_time_ns or None)."""
    from concourse.bass_utils import run_bass_kernel_spmd

    in_maps, meta, use_vb = _prep_in_maps(input_ids, labels, params)
    key = ('prog', use_vb)
    if key not in _CACHE:
        _CACHE[key] = _build_program(use_vb)
    nc = _CACHE[key]

    res = run_bass_kernel_spmd(
        nc, in_maps, core_ids=list(range(N_CORES)), trace=trace)

    logits = np.empty((B, T, V), np.float32)
    S = np.zeros((B, T), np.float64)
    for core in range(N_CORES):
        md = meta[core]
        g, v0, vw = md['g'], md['v0'], md['vw']
        lg = res.results[core]['logits'].reshape(T, VLOC)
        logits[g, :, v0:v0 + vw] = lg[:, :vw]
        se = res.results[core]['sumexp']          # [P, TT]
        S[g] += se.T.reshape(T).astype(np.float64) - md['padcount']

    labels = np.asarray(labels)
    logz = np.log(S[:, :T - 1]).astype(np.float32)
    lab = labels[:, 1:]
    pick = np.take_along_axis(logits[:, :T - 1, :], lab[..., None], axis=-1)[..., 0]
    valid = lab != -100
    nll = np.where(valid, logz - pick, 0.0)
    loss = np.float32(nll.sum() / max(valid.sum(), 1))
    return logits, loss, res.exec_time_ns


def kernel(input_ids, labels, params):
    logits, loss, _ = run_on_device(input_ids, labels, params, trace=False)
    return logits, loss


# revision 21
# speedup vs baseline: 1.0327x; 1.0327x over previous
"""Trainium2 Bass kernel for a 24-layer GPT-2-medium-style LM (tied lm_head + CE loss).

Sharding: DP-2 over batch x TP-4 (megatron) within each half of the 8 cores.
  - group g = cores [4g .. 4g+3] handles batch sample g (T=1024 tokens).
  - within a group, rank r shards: attention heads (4/core), FFN inner dim
    (1024/core), and the tied vocab lm_head (12800 padded cols/core).
  - 2 AllReduces per layer (attn proj partial, fc proj partial) over each
    group of 4 cores.

On-chip layout: activations kept feature-major x_fm[d_part, d_sub, token] so
every linear contracts d on partitions.  LayerNorm mean/sumsq are computed
with ones-vector matmuls (partition reductions) in float32r; per-token scale/
shift vectors are broadcast across partitions with rank-1 matmuls.  Softmax
skips max-subtraction (|scores| < 4) and folds the normalizer into the AV
matmul via an appended ones-column on V; the reciprocal is applied on PSUM
eviction.  LN gains/biases are folded into the adjacent weights on the host.

kernel(**inputs) takes the FULL inputs (as produced by setup_inputs) and
returns (logits [2,1024,50257] fp32, loss fp32) like the reference.
"""

import numpy as np
import ml_dtypes

# model dims (hardcoded for this problem)
V, D, H, HD, L, FF = 50257, 1024, 16, 64, 24, 4096
B, T = 2, 1024
EPS = 1e-5

P = 128
DS = D // P            # 8 feature subtiles
N_CORES = 8
TPG = 4                # tensor-parallel group size
NH = H // TPG          # 4 local heads
QC = NH * HD           # 256 local q/k/v cols
MS = QC // P           # 2 subtiles of q/k/y
FFL = FF // TPG        # 1024 local ffn cols
FS = FFL // P          # 8
TT = T // P            # 8 token tiles
CH = 512               # token chunk (matmul moving free dim)
NCH = T // CH          # 2
VLOC = 12800           # padded vocab shard per core (4*12800 = 51200 >= V)
VT = VLOC // CH        # 25 vocab tiles
RG = [[0, 1, 2, 3], [4, 5, 6, 7]]

_CACHE = {}


def _bf16(x):
    return np.ascontiguousarray(np.asarray(x, np.float32).astype(ml_dtypes.bfloat16))


def _f32(x):
    return np.ascontiguousarray(np.asarray(x, np.float32))


def _fm(arr):
    """[D, M] -> lhsT/rhs SBUF layout [P, D//P, M] with d = ds*P + p."""
    d, m = arr.shape
    return np.ascontiguousarray(arr.reshape(d // P, P, m).transpose(1, 0, 2))


def _percol(vec):
    """[n*P] -> per-partition bias layout [P, n] with col index = ns*P + p."""
    n = vec.shape[0] // P
    return np.ascontiguousarray(vec.reshape(n, P).T)


def _build_program(use_vb):
    import concourse.bass as bass  # noqa: F401
    import concourse.tile as tile
    from concourse import bacc, mybir
    from contextlib import ExitStack

    dt = mybir.dt
    Alu = mybir.AluOpType
    Act = mybir.ActivationFunctionType

    nc = bacc.Bacc(None, target_bir_lowering=False)

    # ---- per-core DRAM I/O ----
    x0_d = nc.dram_tensor("x0", [P, DS, T], dt.float32, kind="ExternalInput")
    wq_d = nc.dram_tensor("wq", [L, P, DS, QC], dt.bfloat16, kind="ExternalInput")
    wk_d = nc.dram_tensor("wk", [L, P, DS, QC], dt.bfloat16, kind="ExternalInput")
    wv_d = nc.dram_tensor("wv", [L, P, DS, QC], dt.bfloat16, kind="ExternalInput")
    bq_d = nc.dram_tensor("bq", [L, P, MS], dt.float32, kind="ExternalInput")
    bk_d = nc.dram_tensor("bk", [L, P, MS], dt.float32, kind="ExternalInput")
    bv_d = nc.dram_tensor("bv", [L, 1, QC], dt.bfloat16, kind="ExternalInput")
    wp_d = nc.dram_tensor("wp", [L, P, MS, D], dt.bfloat16, kind="ExternalInput")
    pb_d = nc.dram_tensor("pb", [L, P, DS], dt.float32, kind="ExternalInput")
    wfc_d = nc.dram_tensor("wfc", [L, FS, P, DS, P], dt.bfloat16, kind="ExternalInput")
    fcb_d = nc.dram_tensor("fcb", [L, P, FS], dt.float32, kind="ExternalInput")
    wfp_d = nc.dram_tensor("wfp", [L, DS, P, FS, P], dt.bfloat16, kind="ExternalInput")
    fpb_d = nc.dram_tensor("fpb", [L, P, DS], dt.float32, kind="ExternalInput")
    wvoc_d = nc.dram_tensor("wvoc", [VT, P, DS, CH], dt.bfloat16, kind="ExternalInput")
    vb_d = nc.dram_tensor("vb", [1, VLOC], dt.bfloat16, kind="ExternalInput")
    mask_d = nc.dram_tensor("mask", [P, P], dt.bfloat16, kind="ExternalInput")

    logits_d = nc.dram_tensor("logits", [TT, P, VLOC], dt.float32, kind="ExternalOutput")
    sumexp_d = nc.dram_tensor("sumexp", [P, TT], dt.float32, kind="ExternalOutput")

    with tile.TileContext(nc) as tc, ExitStack() as ctx:
        const = ctx.enter_context(tc.tile_pool(name="const", bufs=1))
        pers = ctx.enter_context(tc.tile_pool(name="pers", bufs=1))
        wpool = ctx.enter_context(tc.tile_pool(name="w", bufs=1))
        wvpool = ctx.enter_context(tc.tile_pool(name="wv2", bufs=2))
        work = ctx.enter_context(tc.tile_pool(name="work", bufs=3))
        work2 = ctx.enter_context(tc.tile_pool(name="work2", bufs=2))
        tiny1 = ctx.enter_context(tc.tile_pool(name="tiny1", bufs=1))
        tiny2 = ctx.enter_context(tc.tile_pool(name="tiny2", bufs=2))
        wstream = ctx.enter_context(tc.tile_pool(name="wstream", bufs=3))
        bcp = ctx.enter_context(tc.tile_pool(name="bcp", bufs=2))
        zp = ctx.enter_context(tc.tile_pool(name="zp", bufs=2))
        ps = ctx.enter_context(tc.tile_pool(name="ps", bufs=4, space="PSUM"))
        pstat = ctx.enter_context(tc.tile_pool(name="pstat", bufs=2, space="PSUM"))
        pbc = ctx.enter_context(tc.tile_pool(name="pbc", bufs=2, space="PSUM"))
        dram = ctx.enter_context(tc.tile_pool(name="dram", bufs=2, space="DRAM"))
        dramo = ctx.enter_context(tc.tile_pool(name="dramo", bufs=1, space="DRAM"))

        # constants
        ones_col_f = const.tile([P, 1], dt.float32)
        nc.vector.memset(ones_col_f[:], 1.0)
        ones_col_r = const.tile([P, 1], dt.float32r)
        nc.vector.tensor_copy(ones_col_r[:], ones_col_f[:])
        ones_col_b = const.tile([P, 1], dt.bfloat16)
        nc.vector.memset(ones_col_b[:], 1.0)
        ones_row_f = const.tile([1, P], dt.float32)
        nc.vector.memset(ones_row_f[:], 1.0)
        ones_row_r = const.tile([1, P], dt.float32r)
        nc.vector.tensor_copy(ones_row_r[:], ones_row_f[:])
        ones_row_b = const.tile([1, P], dt.bfloat16)
        nc.vector.memset(ones_row_b[:], 1.0)
        mask_sb = const.tile([P, P], dt.bfloat16)
        nc.sync.dma_start(mask_sb[:], mask_d[:])
        eps_sb = const.tile([1, 1], dt.float32)
        nc.vector.memset(eps_sb[:], EPS)

        # persistent activations
        x_fm = pers.tile([P, DS, T], dt.float32)
        h_sb = pers.tile([P, DS, T], dt.bfloat16)
        q_fm = pers.tile([P, MS, T], dt.bfloat16)
        k_fm = pers.tile([P, MS, T], dt.bfloat16)
        v_aug = pers.tile([P, TT, NH, HD + 1], dt.bfloat16)
        y_fm = pers.tile([P, MS, T], dt.bfloat16)
        ffn_sb = pers.tile([P, FS, T], dt.bfloat16)
        accs = pers.tile([P, TT, VT], dt.float32)
        sumexp_sb = pers.tile([P, TT], dt.float32)

        nc.vector.memset(v_aug[:, :, :, HD:HD + 1], 1.0)
        nc.sync.dma_start(x_fm[:], x0_d[:])

        def layernorm():
            """h_sb = LN(x_fm) without gain/bias (folded into weights), bf16."""
            for c in range(NCH):
                tcs = slice(c * CH, (c + 1) * CH)
                p_sum = pstat.tile([1, CH], dt.float32, tag="pstat")
                for d_i in range(DS):
                    xr = work.tile([P, CH], dt.float32r, tag="xr")
                    nc.vector.tensor_copy(xr[:], x_fm[:, d_i, tcs])
                    nc.tensor.matmul(
                        p_sum[:], ones_col_r[:], xr[:],
                        start=d_i == 0, stop=d_i == DS - 1,
                    )
                mu_t = tiny1.tile([1, CH], dt.float32, tag="mu")
                nc.vector.tensor_scalar(
                    mu_t[:], p_sum[:], 1.0 / D, None, Alu.mult)
                p_sq = pstat.tile([1, CH], dt.float32, tag="pstat")
                for d_i in range(DS):
                    xsq = work.tile([P, CH], dt.bfloat16, tag="xsq")
                    nc.scalar.activation(xsq[:], x_fm[:, d_i, tcs], Act.Square)
                    nc.tensor.matmul(
                        p_sq[:], ones_col_b[:], xsq[:],
                        start=d_i == 0, stop=d_i == DS - 1,
                    )
                musq = tiny1.tile([1, CH], dt.float32, tag="musq")
                nc.vector.tensor_tensor(
                    musq[:], mu_t[:], mu_t[:], Alu.mult)
                var = tiny1.tile([1, CH], dt.float32, tag="var")
                nc.vector.scalar_tensor_tensor(
                    var[:], p_sq[:], 1.0 / D, musq[:], Alu.mult, Alu.subtract)
                std = tiny1.tile([1, CH], dt.float32, tag="std")
                nc.scalar.activation(std[:], var[:], Act.Sqrt, bias=eps_sb[0:1, 0:1])
                inv_t = tiny1.tile([1, CH], dt.float32r, tag="inv")
                with nc.allow_low_precision(reason="f32r rounding for broadcast matmul"):
                    nc.vector.reciprocal(inv_t[:], std[:])
                csc_t = tiny1.tile([1, CH], dt.float32r, tag="csc")
                nc.vector.scalar_tensor_tensor(
                    csc_t[:], mu_t[:], -1.0, inv_t[:],
                    Alu.mult, Alu.mult)
                # broadcast a = inv, c = -mu*inv across partitions
                p_a = pbc.tile([P, CH], dt.float32, tag="pbc")
                nc.tensor.matmul(
                    p_a[:], ones_row_r[:], inv_t[:],
                    start=True, stop=True)
                a_bc = bcp.tile([P, CH], dt.float32, tag="abc")
                nc.vector.tensor_copy(a_bc[:], p_a[:])
                p_c = pbc.tile([P, CH], dt.float32, tag="pbc")
                nc.tensor.matmul(
                    p_c[:], ones_row_r[:], csc_t[:],
                    start=True, stop=True)
                c_bc = bcp.tile([P, CH], dt.float32, tag="cbc")
                nc.vector.tensor_copy(c_bc[:], p_c[:])
                for d_i in range(DS):
                    tmp = work2.tile([P, CH], dt.float32, tag="lntmp")
                    nc.vector.tensor_tensor(
                        tmp[:], x_fm[:, d_i, tcs], a_bc[:], Alu.mult)
                    nc.vector.tensor_tensor(
                        h_sb[:, d_i, tcs], tmp[:], c_bc[:], Alu.add)

        def residual_add_chunk(bout_c, bias_sb, tcs):
            """x_fm[:, :, tcs] += AR_result_chunk + bias (bias per out-col)."""
            for n_i in range(DS):
                zs = zp.tile([P, CH], dt.float32, tag="zr")
                nc.sync.dma_start(zs[:], bout_c[:, n_i, :])
                nc.vector.scalar_tensor_tensor(
                    x_fm[:, n_i, tcs], zs[:], bias_sb[:, n_i:n_i + 1],
                    x_fm[:, n_i, tcs], Alu.add, Alu.add)

        for li in range(L):
            # ---- load layer weights ----
            wq_sb = wpool.tile([P, DS, QC], dt.bfloat16, tag="wq")
            nc.sync.dma_start(wq_sb[:], wq_d[li])
            wk_sb = wpool.tile([P, DS, QC], dt.bfloat16, tag="wk")
            nc.sync.dma_start(wk_sb[:], wk_d[li])
            wv_sb = wpool.tile([P, DS, QC], dt.bfloat16, tag="wv")
            nc.sync.dma_start(wv_sb[:], wv_d[li])
            bq_sb = wpool.tile([P, MS], dt.float32, tag="bq")
            nc.sync.dma_start(bq_sb[:], bq_d[li])
            bk_sb = wpool.tile([P, MS], dt.float32, tag="bk")
            nc.sync.dma_start(bk_sb[:], bk_d[li])
            bv_sb = wpool.tile([1, QC], dt.bfloat16, tag="bv")
            nc.sync.dma_start(bv_sb[:], bv_d[li])
            wp_sb = wpool.tile([P, MS, D], dt.bfloat16, tag="wp")
            nc.sync.dma_start(wp_sb[:], wp_d[li])
            pb_sb = wpool.tile([P, DS], dt.float32, tag="pb")
            nc.sync.dma_start(pb_sb[:], pb_d[li])
            fcb_sb = wpool.tile([P, FS], dt.float32, tag="fcb")
            nc.sync.dma_start(fcb_sb[:], fcb_d[li])
            fpb_sb = wpool.tile([P, DS], dt.float32, tag="fpb")
            nc.sync.dma_start(fpb_sb[:], fpb_d[li])

            # ---- ln1 + qkv ----
            layernorm()
            for m_i in range(MS):
                mslice = slice(m_i * P, (m_i + 1) * P)
                for c in range(NCH):
                    tcs = slice(c * CH, (c + 1) * CH)
                    pq = ps.tile([P, CH], dt.float32, tag="ps")
                    for d_i in range(DS):
                        nc.tensor.matmul(
                            pq[:], wq_sb[:, d_i, mslice], h_sb[:, d_i, tcs],
                            start=d_i == 0, stop=d_i == DS - 1)
                    nc.vector.tensor_scalar(
                        q_fm[:, m_i, tcs], pq[:], bq_sb[:, m_i:m_i + 1], None,
                        Alu.add)
                    pk = ps.tile([P, CH], dt.float32, tag="ps")
                    for d_i in range(DS):
                        nc.tensor.matmul(
                            pk[:], wk_sb[:, d_i, mslice], h_sb[:, d_i, tcs],
                            start=d_i == 0, stop=d_i == DS - 1)
                    nc.vector.tensor_scalar(
                        k_fm[:, m_i, tcs], pk[:], bk_sb[:, m_i:m_i + 1], None,
                        Alu.add)
            use_bv = True  # bias-in-psum matmul for token-major V
            for t_i in range(TT):
                tts = slice(t_i * P, (t_i + 1) * P)
                pv = ps.tile([P, QC], dt.float32, tag="ps")
                for d_i in range(DS):
                    nc.tensor.matmul(
                        pv[:], h_sb[:, d_i, tts], wv_sb[:, d_i, :],
                        start=d_i == 0, stop=(not use_bv) and d_i == DS - 1)
                if use_bv:
                    nc.tensor.matmul(
                        pv[:], ones_row_b[:], bv_sb[0:1, :],
                        start=False, stop=True)
                nc.vector.tensor_copy(
                    v_aug[:, t_i, :, 0:HD],
                    pv[:].rearrange("p (h d) -> p h d", h=NH))

            # ---- attention ----
            for qc_ in range(NCH):
                for hh in range(NH):
                    po = (hh % 2) * HD
                    m_i = hh // 2
                    tq0 = qc_ * CH
                    ktiles = list(range(min(TT, qc_ * 4 + 4)))
                    py = ps.tile([HD, CH], dt.float32, tag="ps")
                    pn = pstat.tile([1, CH], dt.float32, tag="pstat")
                    for idx, tkt in enumerate(ktiles):
                        pscr = ps.tile([P, CH], dt.float32, tag="ps")
                        nc.tensor.matmul(
                            pscr[:],
                            k_fm[po:po + HD, m_i, tkt * P:(tkt + 1) * P],
                            q_fm[po:po + HD, m_i, tq0:tq0 + CH],
                            start=True, stop=True)
                        att = work.tile([P, CH], dt.bfloat16, tag="att")
                        nc.scalar.activation(att[:], pscr[:], Act.Exp)
                        o = tkt - qc_ * 4
                        if o >= 0:
                            if o > 0:
                                nc.vector.memset(att[:, 0:o * P], 0.0)
                            nc.vector.tensor_tensor(
                                att[:, o * P:(o + 1) * P],
                                att[:, o * P:(o + 1) * P],
                                mask_sb[:], Alu.mult)
                        nc.tensor.matmul(
                            py[:], v_aug[:, tkt, hh, 0:HD], att[:],
                            start=idx == 0, stop=idx == len(ktiles) - 1)
                        nc.tensor.matmul(
                            pn[:], v_aug[:, tkt, hh, HD:HD + 1], att[:],
                            start=idx == 0, stop=idx == len(ktiles) - 1)
                    rcp = tiny2.tile([1, CH], dt.float32r, tag="rcp")
                    with nc.allow_low_precision(reason="f32r rounding for broadcast matmul"):
                        nc.vector.reciprocal(rcp[:], pn[:])
                    p_r = pbc.tile([HD, CH], dt.float32, tag="pbc")
                    nc.tensor.matmul(
                        p_r[:], ones_row_r[0:1, 0:HD],
                        rcp[:], start=True, stop=True)
                    rbc = bcp.tile([HD, CH], dt.float32, tag="rbc")
                    nc.vector.tensor_copy(rbc[:], p_r[:])
                    if po == 0:
                        nc.vector.tensor_tensor(
                            y_fm[0:HD, m_i, tq0:tq0 + CH], py[0:HD, :], rbc[:],
                            Alu.mult)
                    else:
                        yst = work.tile([HD, CH], dt.bfloat16, tag="yst")
                        nc.vector.tensor_tensor(
                            yst[:], py[0:HD, :], rbc[:], Alu.mult)
                        nc.sync.dma_start(y_fm[po:po + HD, m_i, tq0:tq0 + CH], yst[:])

            # ---- attn proj + chunked AllReduce + residual ----
            for c in range(NCH):
                tcs = slice(c * CH, (c + 1) * CH)
                bin_a = dram.tile([P, DS, CH], dt.float32, tag="arin")
                bout_a = dramo.tile([P, DS, CH], dt.float32, tag="arout")
                for n_i in range(DS):
                    pz = ps.tile([P, CH], dt.float32, tag="ps")
                    for k_i in range(MS):
                        nc.tensor.matmul(
                            pz[:], wp_sb[:, k_i, n_i * P:(n_i + 1) * P],
                            y_fm[:, k_i, tcs],
                            start=k_i == 0, stop=k_i == MS - 1)
                    zev = work2.tile([P, CH], dt.float32, tag="zev")
                    nc.vector.tensor_copy(zev[:], pz[:])
                    nc.sync.dma_start(bin_a[:, n_i, :], zev[:])
                nc.gpsimd.collective_compute(
                    "AllReduce", Alu.add, replica_groups=RG,
                    ins=[bin_a.opt()], outs=[bout_a.opt()])
                residual_add_chunk(bout_a, pb_sb, tcs)

            # ---- ln2 + ffn ----
            layernorm()
            for f_i in range(FS):
                fslice = slice(f_i * P, (f_i + 1) * P)
                wfct = wstream.tile([P, DS, P], dt.bfloat16, tag="wfc")
                nc.sync.dma_start(wfct[:], wfc_d[li, f_i])
                for c in range(NCH):
                    tcs = slice(c * CH, (c + 1) * CH)
                    pf = ps.tile([P, CH], dt.float32, tag="ps")
                    for d_i in range(DS):
                        nc.tensor.matmul(
                            pf[:], wfct[:, d_i, :], h_sb[:, d_i, tcs],
                            start=d_i == 0, stop=d_i == DS - 1)
                    nc.scalar.activation(
                        ffn_sb[:, f_i, tcs], pf[:], Act.Gelu,
                        bias=fcb_sb[:, f_i:f_i + 1])
            for c in range(NCH):
                tcs = slice(c * CH, (c + 1) * CH)
                bin_b = dram.tile([P, DS, CH], dt.float32, tag="arin2")
                bout_b = dramo.tile([P, DS, CH], dt.float32, tag="arout2")
                for n_i in range(DS):
                    wfpt = wstream.tile([P, FS, P], dt.bfloat16, tag="wfp")
                    nc.sync.dma_start(wfpt[:], wfp_d[li, n_i])
                    pz = ps.tile([P, CH], dt.float32, tag="ps")
                    for f_i in range(FS):
                        nc.tensor.matmul(
                            pz[:], wfpt[:, f_i, :],
                            ffn_sb[:, f_i, tcs],
                            start=f_i == 0, stop=f_i == FS - 1)
                    zev = work2.tile([P, CH], dt.float32, tag="zev")
                    nc.vector.tensor_copy(zev[:], pz[:])
                    nc.sync.dma_start(bin_b[:, n_i, :], zev[:])
                nc.gpsimd.collective_compute(
                    "AllReduce", Alu.add, replica_groups=RG,
                    ins=[bin_b.opt()], outs=[bout_b.opt()])
                residual_add_chunk(bout_b, fpb_sb, tcs)

        # ---- final LN + lm_head + sumexp stats ----
        layernorm()
        vb_sb = None
        if use_vb:
            vb_sb = const.tile([1, VLOC], dt.bfloat16)
            nc.sync.dma_start(vb_sb[:], vb_d[:])
        for vc in range(VT):
            vcs = slice(vc * CH, (vc + 1) * CH)
            wvc = wvpool.tile([P, DS, CH], dt.bfloat16, tag="wvoc")
            nc.sync.dma_start(wvc[:], wvoc_d[vc])
            for t_i in range(TT):
                tts = slice(t_i * P, (t_i + 1) * P)
                pl = ps.tile([P, CH], dt.float32, tag="ps")
                for d_i in range(DS):
                    nc.tensor.matmul(
                        pl[:], h_sb[:, d_i, tts], wvc[:, d_i, :],
                        start=d_i == 0, stop=(not use_vb) and d_i == DS - 1)
                if use_vb:
                    nc.tensor.matmul(
                        pl[:], ones_row_b[:], vb_sb[0:1, vcs],
                        start=False, stop=True)
                lgev = work2.tile([P, CH], dt.float32, tag="lgev")
                nc.vector.tensor_copy(lgev[:], pl[:])
                nc.sync.dma_start(logits_d[t_i, :, vcs], lgev[:])
                esc = work2.tile([P, CH], dt.float32, tag="esc")
                nc.scalar.activation(
                    esc[:], lgev[:], Act.Exp, accum_out=accs[:, t_i, vc:vc + 1])
        for t_i in range(TT):
            nc.vector.tensor_reduce(
                sumexp_sb[:, t_i:t_i + 1], accs[:, t_i, :],
                mybir.AxisListType.X, Alu.add)
        nc.sync.dma_start(sumexp_d[:], sumexp_sb[:])

    nc.compile()
    return nc


def _prep_in_maps(input_ids, labels, params):
    p = {k: _f32(v) for k, v in params.items()}
    ids = np.asarray(input_ids)
    sc = np.float32(1.0 / np.sqrt(HD))

    vb = p['wte'] @ p['lnf_b']          # [V] logit bias from folded lnf_b
    use_vb = bool(np.any(vb != 0.0))

    # per-layer folded weights (shared across cores; sliced per rank)
    wq_l, wk_l, wv_l = [], [], []
    bq_l, bk_l, bv_l = [], [], []
    wp_l, pb_l, wfc_l, fcb_l, wfp_l, fpb_l = [], [], [], [], [], []
    for l in range(L):
        w_eff = p['ln1_g'][l][:, None] * p['attn_w'][l]         # [D, 3D]
        b_eff = p['attn_b'][l] + p['ln1_b'][l] @ p['attn_w'][l]  # [3D]
        w_eff = w_eff.copy()
        b_eff = b_eff.copy()
        w_eff[:, :D] *= sc
        b_eff[:D] *= sc
        wq_l.append(w_eff[:, 0:D])
        wk_l.append(w_eff[:, D:2 * D])
        wv_l.append(w_eff[:, 2 * D:3 * D])
        bq_l.append(b_eff[0:D])
        bk_l.append(b_eff[D:2 * D])
        bv_l.append(b_eff[2 * D:3 * D])
        wp_l.append(p['attn_proj_w'][l])
        pb_l.append(p['attn_proj_b'][l])
        wfc_l.append(p['ln2_g'][l][:, None] * p['fc_w'][l])
        fcb_l.append(p['fc_b'][l] + p['ln2_b'][l] @ p['fc_w'][l])
        wfp_l.append(p['fc_proj_w'][l])
        fpb_l.append(p['fc_proj_b'][l])

    wteT_eff = (p['lnf_g'][:, None] * p['wte'].T)                # [D, V]
    mask_np = _bf16(np.triu(np.ones((P, P), np.float32)))

    in_maps = []
    meta = []
    for core in range(N_CORES):
        g, r = divmod(core, TPG)
        qs = slice(r * QC, (r + 1) * QC)
        fsl = slice(r * FFL, (r + 1) * FFL)
        v0 = r * VLOC
        vw = max(0, min(VLOC, V - v0))       # real vocab width of this shard
        wvoc = np.zeros((D, VLOC), np.float32)
        wvoc[:, :vw] = wteT_eff[:, v0:v0 + vw]
        vb_core = np.full((VLOC,), -1e4 if use_vb else 0.0, np.float32)
        vb_core[:vw] = vb[v0:v0 + vw]

        x = p['wte'][ids[g]] + p['wpe'][:T]          # [T, D]
        m = {
            'x0': _f32(_fm(x.T)),
            'wq': _bf16(np.stack([_fm(wq_l[l][:, qs]) for l in range(L)])),
            'wk': _bf16(np.stack([_fm(wk_l[l][:, qs]) for l in range(L)])),
            'wv': _bf16(np.stack([_fm(wv_l[l][:, qs]) for l in range(L)])),
            'bq': _f32(np.stack([_percol(bq_l[l][qs]) for l in range(L)])),
            'bk': _f32(np.stack([_percol(bk_l[l][qs]) for l in range(L)])),
            'bv': _bf16(np.stack([bv_l[l][qs][None, :] for l in range(L)])),
            'wp': _bf16(np.stack([_fm(wp_l[l][qs, :]) for l in range(L)])),
            'pb': _f32(np.stack([_percol(pb_l[l]) for l in range(L)])),
            'wfc': _bf16(np.stack([
                np.stack([_fm(wfc_l[l][:, fsl])[:, :, f * P:(f + 1) * P]
                          for f in range(FS)]) for l in range(L)])),
            'fcb': _f32(np.stack([_percol(fcb_l[l][fsl]) for l in range(L)])),
            'wfp': _bf16(np.stack([
                np.stack([_fm(wfp_l[l][fsl, :])[:, :, n * P:(n + 1) * P]
                          for n in range(DS)]) for l in range(L)])),
            'fpb': _f32(np.stack([_percol(fpb_l[l]) for l in range(L)])),
            'wvoc': _bf16(np.stack([_fm(wvoc)[:, :, vc * CH:(vc + 1) * CH]
                                    for vc in range(VT)])),
            'vb': _bf16(vb_core[None, :]),
            'mask': mask_np,
        }
        in_maps.append(m)
        meta.append({'g': g, 'r': r, 'v0': v0, 'vw': vw,
                     'padcount': 0 if use_vb else (VLOC - vw)})
    return in_maps, meta, use_vb


def run_on_device(input_ids, labels, params, trace=False):
    """Returns (logits [B,T,V] f32, loss f32, exec_time_ns or None)."""
    from concourse.bass_utils import run_bass_kernel_spmd

    in_maps, meta, use_vb = _prep_in_maps(input_ids, labels, params)
    key = ('prog', use_vb)
    if key not in _CACHE:
        _CACHE[key] = _build_program(use_vb)
    nc = _CACHE[key]

    res = run_bass_kernel_spmd(
        nc, in_maps, core_ids=list(range(N_CORES)), trace=trace)

    logits = np.empty((B, T, V), np.float32)
    S = np.zeros((B, T), np.float64)
    for core in range(N_CORES):
        md = meta[core]
        g, v0, vw = md['g'], md['v0'], md['vw']
        lg = res.results[core]['logits'].reshape(T, VLOC)
        logits[g, :, v0:v0 + vw] = lg[:, :vw]
        se = res.results[core]['sumexp']          # [P, TT]
        S[g] += se.T.reshape(T).astype(np.float64) - md['padcount']

    labels = np.asarray(labels)
    logz = np.log(S[:, :T - 1]).astype(np.float32)
    lab = labels[:, 1:]
    pick = np.take_along_axis(logits[:, :T - 1, :], lab[..., None], axis=-1)[..., 0]
    valid = lab != -100
    nll = np.where(valid, logz - pick, 0.0)
    loss = np.float32(nll.sum() / max(valid.sum(), 1))
    return logits, loss, res.exec_time_ns


def kernel(input_ids, labels, params):
    logits, loss, _ = run_on_device(input_ids, labels, params, trace=False)
    return logits, loss


# revision 22
# speedup vs baseline: 1.2617x; 1.2217x over previous
"""Trainium2 Bass kernel for a 24-layer GPT-2-medium-style LM (tied lm_head + CE loss).

Sharding: DP-2 over batch x TP-4 (megatron) within each half of the 8 cores.
  - group g = cores [4g .. 4g+3] handles batch sample g (T=1024 tokens).
  - within a group, rank r shards: attention heads (4/core), FFN inner dim
    (1024/core), and the tied vocab lm_head (12800 padded cols/core).
  - 2 AllReduces per layer (attn proj partial, fc proj partial) over each
    group of 4 cores.

On-chip layout: activations kept feature-major x_fm[d_part, d_sub, token] so
every linear contracts d on partitions.  LayerNorm mean/sumsq are computed
with ones-vector matmuls (partition reductions) in float32r; per-token scale/
shift vectors are broadcast across partitions with rank-1 matmuls.  Softmax
skips max-subtraction (|scores| < 4) and folds the normalizer into the AV
matmul via an appended ones-column on V; the reciprocal is applied on PSUM
eviction.  LN gains/biases are folded into the adjacent weights on the host.

kernel(**inputs) takes the FULL inputs (as produced by setup_inputs) and
returns (logits [2,1024,50257] fp32, loss fp32) like the reference.
"""

import numpy as np
import ml_dtypes

# model dims (hardcoded for this problem)
V, D, H, HD, L, FF = 50257, 1024, 16, 64, 24, 4096
B, T = 2, 1024
EPS = 1e-5

P = 128
DS = D // P            # 8 feature subtiles
N_CORES = 8
TPG = 4                # tensor-parallel group size
NH = H // TPG          # 4 local heads
QC = NH * HD           # 256 local q/k/v cols
MS = QC // P           # 2 subtiles of q/k/y
FFL = FF // TPG        # 1024 local ffn cols
FS = FFL // P          # 8
TT = T // P            # 8 token tiles
CH = 512               # token chunk (matmul moving free dim)
NCH = T // CH          # 2
VLOC = 12800           # padded vocab shard per core (4*12800 = 51200 >= V)
VT = VLOC // CH        # 25 vocab tiles
RG = [[0, 1, 2, 3], [4, 5, 6, 7]]

_CACHE = {}


def _bf16(x):
    return np.ascontiguousarray(np.asarray(x, np.float32).astype(ml_dtypes.bfloat16))


def _f32(x):
    return np.ascontiguousarray(np.asarray(x, np.float32))


def _fm(arr):
    """[D, M] -> lhsT/rhs SBUF layout [P, D//P, M] with d = ds*P + p."""
    d, m = arr.shape
    return np.ascontiguousarray(arr.reshape(d // P, P, m).transpose(1, 0, 2))


def _percol(vec):
    """[n*P] -> per-partition bias layout [P, n] with col index = ns*P + p."""
    n = vec.shape[0] // P
    return np.ascontiguousarray(vec.reshape(n, P).T)


def _build_program(use_vb):
    import concourse.bass as bass  # noqa: F401
    import concourse.tile as tile
    from concourse import bacc, mybir
    from contextlib import ExitStack

    dt = mybir.dt
    Alu = mybir.AluOpType
    Act = mybir.ActivationFunctionType

    nc = bacc.Bacc(None, target_bir_lowering=False)

    # ---- per-core DRAM I/O ----
    x0_d = nc.dram_tensor("x0", [P, DS, T], dt.float32, kind="ExternalInput")
    wq_d = nc.dram_tensor("wq", [L, P, DS, QC], dt.bfloat16, kind="ExternalInput")
    wk_d = nc.dram_tensor("wk", [L, P, DS, QC], dt.bfloat16, kind="ExternalInput")
    wv_d = nc.dram_tensor("wv", [L, P, DS, QC], dt.bfloat16, kind="ExternalInput")
    bq_d = nc.dram_tensor("bq", [L, P, MS], dt.float32, kind="ExternalInput")
    bk_d = nc.dram_tensor("bk", [L, P, MS], dt.float32, kind="ExternalInput")
    bv_d = nc.dram_tensor("bv", [L, 1, QC], dt.bfloat16, kind="ExternalInput")
    wp_d = nc.dram_tensor("wp", [L, P, MS, D], dt.bfloat16, kind="ExternalInput")
    pb_d = nc.dram_tensor("pb", [L, P, DS], dt.float32, kind="ExternalInput")
    wfc_d = nc.dram_tensor("wfc", [L, FS, P, DS, P], dt.bfloat16, kind="ExternalInput")
    fcb_d = nc.dram_tensor("fcb", [L, P, FS], dt.float32, kind="ExternalInput")
    wfp_d = nc.dram_tensor("wfp", [L, DS, P, FS, P], dt.bfloat16, kind="ExternalInput")
    fpb_d = nc.dram_tensor("fpb", [L, P, DS], dt.float32, kind="ExternalInput")
    wvoc_d = nc.dram_tensor("wvoc", [VT, P, DS, CH], dt.bfloat16, kind="ExternalInput")
    vb_d = nc.dram_tensor("vb", [1, VLOC], dt.bfloat16, kind="ExternalInput")
    mask_d = nc.dram_tensor("mask", [P, P], dt.bfloat16, kind="ExternalInput")

    logits_d = nc.dram_tensor("logits", [TT, P, VLOC], dt.float32, kind="ExternalOutput")
    sumexp_d = nc.dram_tensor("sumexp", [P, TT], dt.float32, kind="ExternalOutput")

    with tile.TileContext(nc) as tc, ExitStack() as ctx:
        const = ctx.enter_context(tc.tile_pool(name="const", bufs=1))
        pers = ctx.enter_context(tc.tile_pool(name="pers", bufs=1))
        wpool = ctx.enter_context(tc.tile_pool(name="w", bufs=1))
        wvpool = ctx.enter_context(tc.tile_pool(name="wv2", bufs=2))
        work = ctx.enter_context(tc.tile_pool(name="work", bufs=3))
        work2 = ctx.enter_context(tc.tile_pool(name="work2", bufs=2))
        tiny1 = ctx.enter_context(tc.tile_pool(name="tiny1", bufs=1))
        tiny2 = ctx.enter_context(tc.tile_pool(name="tiny2", bufs=2))
        wstream = ctx.enter_context(tc.tile_pool(name="wstream", bufs=3))
        bcp = ctx.enter_context(tc.tile_pool(name="bcp", bufs=2))
        zp = ctx.enter_context(tc.tile_pool(name="zp", bufs=2))
        ps = ctx.enter_context(tc.tile_pool(name="ps", bufs=4, space="PSUM"))
        pstat = ctx.enter_context(tc.tile_pool(name="pstat", bufs=2, space="PSUM"))
        pbc = ctx.enter_context(tc.tile_pool(name="pbc", bufs=2, space="PSUM"))
        dram = ctx.enter_context(tc.tile_pool(name="dram", bufs=2, space="DRAM"))
        dramo = ctx.enter_context(tc.tile_pool(name="dramo", bufs=1, space="DRAM"))

        # constants
        ones_col_f = const.tile([P, 1], dt.float32)
        nc.vector.memset(ones_col_f[:], 1.0)
        ones_col_r = const.tile([P, 1], dt.float32r)
        nc.vector.tensor_copy(ones_col_r[:], ones_col_f[:])
        ones_col_b = const.tile([P, 1], dt.bfloat16)
        nc.vector.memset(ones_col_b[:], 1.0)
        ones_row_f = const.tile([1, P], dt.float32)
        nc.vector.memset(ones_row_f[:], 1.0)
        ones_row_r = const.tile([1, P], dt.float32r)
        nc.vector.tensor_copy(ones_row_r[:], ones_row_f[:])
        ones_row_b = const.tile([1, P], dt.bfloat16)
        nc.vector.memset(ones_row_b[:], 1.0)
        mask_sb = const.tile([P, P], dt.bfloat16)
        nc.sync.dma_start(mask_sb[:], mask_d[:])
        eps_sb = const.tile([1, 1], dt.float32)
        nc.vector.memset(eps_sb[:], EPS)

        # persistent activations
        x_fm = pers.tile([P, DS, T], dt.float32)
        h_sb = pers.tile([P, DS, T], dt.bfloat16)
        q_fm = pers.tile([P, MS, T], dt.bfloat16)
        k_fm = pers.tile([P, MS, T], dt.bfloat16)
        v_aug = pers.tile([P, TT, NH, HD + 1], dt.bfloat16)
        y_fm = pers.tile([P, MS, T], dt.bfloat16)
        ffn_sb = pers.tile([P, FS, T], dt.bfloat16)
        accs = pers.tile([P, TT, VT], dt.float32)
        sumexp_sb = pers.tile([P, TT], dt.float32)

        nc.vector.memset(v_aug[:, :, :, HD:HD + 1], 1.0)
        nc.sync.dma_start(x_fm[:], x0_d[:])

        def layernorm():
            """h_sb = LN(x_fm) without gain/bias (folded into weights), bf16."""
            for c in range(NCH):
                tcs = slice(c * CH, (c + 1) * CH)
                p_sum = pstat.tile([1, CH], dt.float32, tag="pstat")
                for d_i in range(DS):
                    xr = work.tile([P, CH], dt.float32r, tag="xr")
                    nc.vector.tensor_copy(xr[:], x_fm[:, d_i, tcs])
                    nc.tensor.matmul(
                        p_sum[:], ones_col_r[:], xr[:],
                        start=d_i == 0, stop=d_i == DS - 1,
                    )
                mu_t = tiny1.tile([1, CH], dt.float32, tag="mu")
                nc.vector.tensor_scalar(
                    mu_t[:], p_sum[:], 1.0 / D, None, Alu.mult)
                p_sq = pstat.tile([1, CH], dt.float32, tag="pstat")
                for d_i in range(DS):
                    xsq = work.tile([P, CH], dt.bfloat16, tag="xsq")
                    nc.scalar.activation(xsq[:], x_fm[:, d_i, tcs], Act.Square)
                    nc.tensor.matmul(
                        p_sq[:], ones_col_b[:], xsq[:],
                        start=d_i == 0, stop=d_i == DS - 1,
                    )
                musq = tiny1.tile([1, CH], dt.float32, tag="musq")
                nc.vector.tensor_tensor(
                    musq[:], mu_t[:], mu_t[:], Alu.mult)
                var = tiny1.tile([1, CH], dt.float32, tag="var")
                nc.vector.scalar_tensor_tensor(
                    var[:], p_sq[:], 1.0 / D, musq[:], Alu.mult, Alu.subtract)
                std = tiny1.tile([1, CH], dt.float32, tag="std")
                nc.scalar.activation(std[:], var[:], Act.Sqrt, bias=eps_sb[0:1, 0:1])
                inv_t = tiny1.tile([1, CH], dt.float32r, tag="inv")
                with nc.allow_low_precision(reason="f32r rounding for broadcast matmul"):
                    nc.vector.reciprocal(inv_t[:], std[:])
                csc_t = tiny1.tile([1, CH], dt.float32r, tag="csc")
                nc.vector.scalar_tensor_tensor(
                    csc_t[:], mu_t[:], -1.0, inv_t[:],
                    Alu.mult, Alu.mult)
                # broadcast a = inv, c = -mu*inv across partitions
                p_a = pbc.tile([P, CH], dt.float32, tag="pbc")
                nc.tensor.matmul(
                    p_a[:], ones_row_r[:], inv_t[:],
                    start=True, stop=True)
                a_bc = bcp.tile([P, CH], dt.float32, tag="abc")
                nc.vector.tensor_copy(a_bc[:], p_a[:])
                p_c = pbc.tile([P, CH], dt.float32, tag="pbc")
                nc.tensor.matmul(
                    p_c[:], ones_row_r[:], csc_t[:],
                    start=True, stop=True)
                c_bc = bcp.tile([P, CH], dt.float32, tag="cbc")
                nc.vector.tensor_copy(c_bc[:], p_c[:])
                for d_i in range(DS):
                    tmp = work2.tile([P, CH], dt.float32, tag="lntmp")
                    nc.vector.tensor_tensor(
                        tmp[:], x_fm[:, d_i, tcs], a_bc[:], Alu.mult)
                    nc.vector.tensor_tensor(
                        h_sb[:, d_i, tcs], tmp[:], c_bc[:], Alu.add)

        def residual_add_chunk(bout_c, bias_sb, tcs):
            """x_fm[:, :, tcs] += AR_result_chunk + bias (bias per out-col)."""
            for n_i in range(DS):
                zs = zp.tile([P, CH], dt.bfloat16, tag="zr")
                nc.sync.dma_start(zs[:], bout_c[:, n_i, :])
                nc.vector.scalar_tensor_tensor(
                    x_fm[:, n_i, tcs], zs[:], bias_sb[:, n_i:n_i + 1],
                    x_fm[:, n_i, tcs], Alu.add, Alu.add)

        for li in range(L):
            # ---- load layer weights ----
            wq_sb = wpool.tile([P, DS, QC], dt.bfloat16, tag="wq")
            nc.sync.dma_start(wq_sb[:], wq_d[li])
            wk_sb = wpool.tile([P, DS, QC], dt.bfloat16, tag="wk")
            nc.sync.dma_start(wk_sb[:], wk_d[li])
            wv_sb = wpool.tile([P, DS, QC], dt.bfloat16, tag="wv")
            nc.sync.dma_start(wv_sb[:], wv_d[li])
            bq_sb = wpool.tile([P, MS], dt.float32, tag="bq")
            nc.sync.dma_start(bq_sb[:], bq_d[li])
            bk_sb = wpool.tile([P, MS], dt.float32, tag="bk")
            nc.sync.dma_start(bk_sb[:], bk_d[li])
            bv_sb = wpool.tile([1, QC], dt.bfloat16, tag="bv")
            nc.sync.dma_start(bv_sb[:], bv_d[li])
            wp_sb = wpool.tile([P, MS, D], dt.bfloat16, tag="wp")
            nc.sync.dma_start(wp_sb[:], wp_d[li])
            pb_sb = wpool.tile([P, DS], dt.float32, tag="pb")
            nc.sync.dma_start(pb_sb[:], pb_d[li])
            fcb_sb = wpool.tile([P, FS], dt.float32, tag="fcb")
            nc.sync.dma_start(fcb_sb[:], fcb_d[li])
            fpb_sb = wpool.tile([P, DS], dt.float32, tag="fpb")
            nc.sync.dma_start(fpb_sb[:], fpb_d[li])

            # ---- ln1 + qkv ----
            layernorm()
            for m_i in range(MS):
                mslice = slice(m_i * P, (m_i + 1) * P)
                for c in range(NCH):
                    tcs = slice(c * CH, (c + 1) * CH)
                    pq = ps.tile([P, CH], dt.float32, tag="ps")
                    for d_i in range(DS):
                        nc.tensor.matmul(
                            pq[:], wq_sb[:, d_i, mslice], h_sb[:, d_i, tcs],
                            start=d_i == 0, stop=d_i == DS - 1)
                    nc.vector.tensor_scalar(
                        q_fm[:, m_i, tcs], pq[:], bq_sb[:, m_i:m_i + 1], None,
                        Alu.add)
                    pk = ps.tile([P, CH], dt.float32, tag="ps")
                    for d_i in range(DS):
                        nc.tensor.matmul(
                            pk[:], wk_sb[:, d_i, mslice], h_sb[:, d_i, tcs],
                            start=d_i == 0, stop=d_i == DS - 1)
                    nc.vector.tensor_scalar(
                        k_fm[:, m_i, tcs], pk[:], bk_sb[:, m_i:m_i + 1], None,
                        Alu.add)
            use_bv = True  # bias-in-psum matmul for token-major V
            for t_i in range(TT):
                tts = slice(t_i * P, (t_i + 1) * P)
                pv = ps.tile([P, QC], dt.float32, tag="ps")
                for d_i in range(DS):
                    nc.tensor.matmul(
                        pv[:], h_sb[:, d_i, tts], wv_sb[:, d_i, :],
                        start=d_i == 0, stop=(not use_bv) and d_i == DS - 1)
                if use_bv:
                    nc.tensor.matmul(
                        pv[:], ones_row_b[:], bv_sb[0:1, :],
                        start=False, stop=True)
                nc.vector.tensor_copy(
                    v_aug[:, t_i, :, 0:HD],
                    pv[:].rearrange("p (h d) -> p h d", h=NH))

            # ---- attention ----
            for qc_ in range(NCH):
                for hh in range(NH):
                    po = (hh % 2) * HD
                    m_i = hh // 2
                    tq0 = qc_ * CH
                    ktiles = list(range(min(TT, qc_ * 4 + 4)))
                    py = ps.tile([HD, CH], dt.float32, tag="ps")
                    pn = pstat.tile([1, CH], dt.float32, tag="pstat")
                    for idx, tkt in enumerate(ktiles):
                        pscr = ps.tile([P, CH], dt.float32, tag="ps")
                        nc.tensor.matmul(
                            pscr[:],
                            k_fm[po:po + HD, m_i, tkt * P:(tkt + 1) * P],
                            q_fm[po:po + HD, m_i, tq0:tq0 + CH],
                            start=True, stop=True)
                        att = work.tile([P, CH], dt.bfloat16, tag="att")
                        nc.scalar.activation(att[:], pscr[:], Act.Exp)
                        o = tkt - qc_ * 4
                        if o >= 0:
                            if o > 0:
                                nc.vector.memset(att[:, 0:o * P], 0.0)
                            nc.vector.tensor_tensor(
                                att[:, o * P:(o + 1) * P],
                                att[:, o * P:(o + 1) * P],
                                mask_sb[:], Alu.mult)
                        nc.tensor.matmul(
                            py[:], v_aug[:, tkt, hh, 0:HD], att[:],
                            start=idx == 0, stop=idx == len(ktiles) - 1)
                        nc.tensor.matmul(
                            pn[:], v_aug[:, tkt, hh, HD:HD + 1], att[:],
                            start=idx == 0, stop=idx == len(ktiles) - 1)
                    rcp = tiny2.tile([1, CH], dt.float32r, tag="rcp")
                    with nc.allow_low_precision(reason="f32r rounding for broadcast matmul"):
                        nc.vector.reciprocal(rcp[:], pn[:])
                    p_r = pbc.tile([HD, CH], dt.float32, tag="pbc")
                    nc.tensor.matmul(
                        p_r[:], ones_row_r[0:1, 0:HD],
                        rcp[:], start=True, stop=True)
                    rbc = bcp.tile([HD, CH], dt.float32, tag="rbc")
                    nc.vector.tensor_copy(rbc[:], p_r[:])
                    if po == 0:
                        nc.vector.tensor_tensor(
                            y_fm[0:HD, m_i, tq0:tq0 + CH], py[0:HD, :], rbc[:],
                            Alu.mult)
                    else:
                        yst = work.tile([HD, CH], dt.bfloat16, tag="yst")
                        nc.vector.tensor_tensor(
                            yst[:], py[0:HD, :], rbc[:], Alu.mult)
                        nc.sync.dma_start(y_fm[po:po + HD, m_i, tq0:tq0 + CH], yst[:])

            # ---- attn proj + chunked AllReduce + residual ----
            for c in range(NCH):
                tcs = slice(c * CH, (c + 1) * CH)
                bin_a = dram.tile([P, DS, CH], dt.bfloat16, tag="arin")
                bout_a = dramo.tile([P, DS, CH], dt.bfloat16, tag="arout")
                for n_i in range(DS):
                    pz = ps.tile([P, CH], dt.float32, tag="ps")
                    for k_i in range(MS):
                        nc.tensor.matmul(
                            pz[:], wp_sb[:, k_i, n_i * P:(n_i + 1) * P],
                            y_fm[:, k_i, tcs],
                            start=k_i == 0, stop=k_i == MS - 1)
                    zev = work2.tile([P, CH], dt.bfloat16, tag="zev")
                    nc.vector.tensor_copy(zev[:], pz[:])
                    nc.sync.dma_start(bin_a[:, n_i, :], zev[:])
                nc.gpsimd.collective_compute(
                    "AllReduce", Alu.add, replica_groups=RG,
                    ins=[bin_a.opt()], outs=[bout_a.opt()])
                residual_add_chunk(bout_a, pb_sb, tcs)

            # ---- ln2 + ffn ----
            layernorm()
            for f_i in range(FS):
                fslice = slice(f_i * P, (f_i + 1) * P)
                wfct = wstream.tile([P, DS, P], dt.bfloat16, tag="wfc")
                nc.sync.dma_start(wfct[:], wfc_d[li, f_i])
                for c in range(NCH):
                    tcs = slice(c * CH, (c + 1) * CH)
                    pf = ps.tile([P, CH], dt.float32, tag="ps")
                    for d_i in range(DS):
                        nc.tensor.matmul(
                            pf[:], wfct[:, d_i, :], h_sb[:, d_i, tcs],
                            start=d_i == 0, stop=d_i == DS - 1)
                    nc.scalar.activation(
                        ffn_sb[:, f_i, tcs], pf[:], Act.Gelu,
                        bias=fcb_sb[:, f_i:f_i + 1])
            for c in range(NCH):
                tcs = slice(c * CH, (c + 1) * CH)
                bin_b = dram.tile([P, DS, CH], dt.bfloat16, tag="arin2")
                bout_b = dramo.tile([P, DS, CH], dt.bfloat16, tag="arout2")
                for n_i in range(DS):
                    wfpt = wstream.tile([P, FS, P], dt.bfloat16, tag="wfp")
                    nc.sync.dma_start(wfpt[:], wfp_d[li, n_i])
                    pz = ps.tile([P, CH], dt.float32, tag="ps")
                    for f_i in range(FS):
                        nc.tensor.matmul(
                            pz[:], wfpt[:, f_i, :],
                            ffn_sb[:, f_i, tcs],
                            start=f_i == 0, stop=f_i == FS - 1)
                    zev = work2.tile([P, CH], dt.bfloat16, tag="zev")
                    nc.vector.tensor_copy(zev[:], pz[:])
                    nc.sync.dma_start(bin_b[:, n_i, :], zev[:])
                nc.gpsimd.collective_compute(
                    "AllReduce", Alu.add, replica_groups=RG,
                    ins=[bin_b.opt()], outs=[bout_b.opt()])
                residual_add_chunk(bout_b, fpb_sb, tcs)

        # ---- final LN + lm_head + sumexp stats ----
        layernorm()
        vb_sb = None
        if use_vb:
            vb_sb = const.tile([1, VLOC], dt.bfloat16)
            nc.sync.dma_start(vb_sb[:], vb_d[:])
        for vc in range(VT):
            vcs = slice(vc * CH, (vc + 1) * CH)
            wvc = wvpool.tile([P, DS, CH], dt.bfloat16, tag="wvoc")
            nc.sync.dma_start(wvc[:], wvoc_d[vc])
            for t_i in range(TT):
                tts = slice(t_i * P, (t_i + 1) * P)
                pl = ps.tile([P, CH], dt.float32, tag="ps")
                for d_i in range(DS):
                    nc.tensor.matmul(
                        pl[:], h_sb[:, d_i, tts], wvc[:, d_i, :],
                        start=d_i == 0, stop=(not use_vb) and d_i == DS - 1)
                if use_vb:
                    nc.tensor.matmul(
                        pl[:], ones_row_b[:], vb_sb[0:1, vcs],
                        start=False, stop=True)
                lgev = work2.tile([P, CH], dt.float32, tag="lgev")
                nc.vector.tensor_copy(lgev[:], pl[:])
                nc.sync.dma_start(logits_d[t_i, :, vcs], lgev[:])
                esc = work2.tile([P, CH], dt.float32, tag="esc")
                nc.scalar.activation(
                    esc[:], lgev[:], Act.Exp, accum_out=accs[:, t_i, vc:vc + 1])
        for t_i in range(TT):
            nc.vector.tensor_reduce(
                sumexp_sb[:, t_i:t_i + 1], accs[:, t_i, :],
                mybir.AxisListType.X, Alu.add)
        nc.sync.dma_start(sumexp_d[:], sumexp_sb[:])

    nc.compile()
    return nc


def _prep_in_maps(input_ids, labels, params):
    p = {k: _f32(v) for k, v in params.items()}
    ids = np.asarray(input_ids)
    sc = np.float32(1.0 / np.sqrt(HD))

    vb = p['wte'] @ p['lnf_b']          # [V] logit bias from folded lnf_b
    use_vb = bool(np.any(vb != 0.0))

    # per-layer folded weights (shared across cores; sliced per rank)
    wq_l, wk_l, wv_l = [], [], []
    bq_l, bk_l, bv_l = [], [], []
    wp_l, pb_l, wfc_l, fcb_l, wfp_l, fpb_l = [], [], [], [], [], []
    for l in range(L):
        w_eff = p['ln1_g'][l][:, None] * p['attn_w'][l]         # [D, 3D]
        b_eff = p['attn_b'][l] + p['ln1_b'][l] @ p['attn_w'][l]  # [3D]
        w_eff = w_eff.copy()
        b_eff = b_eff.copy()
        w_eff[:, :D] *= sc
        b_eff[:D] *= sc
        wq_l.append(w_eff[:, 0:D])
        wk_l.append(w_eff[:, D:2 * D])
        wv_l.append(w_eff[:, 2 * D:3 * D])
        bq_l.append(b_eff[0:D])
        bk_l.append(b_eff[D:2 * D])
        bv_l.append(b_eff[2 * D:3 * D])
        wp_l.append(p['attn_proj_w'][l])
        pb_l.append(p['attn_proj_b'][l])
        wfc_l.append(p['ln2_g'][l][:, None] * p['fc_w'][l])
        fcb_l.append(p['fc_b'][l] + p['ln2_b'][l] @ p['fc_w'][l])
        wfp_l.append(p['fc_proj_w'][l])
        fpb_l.append(p['fc_proj_b'][l])

    wteT_eff = (p['lnf_g'][:, None] * p['wte'].T)                # [D, V]
    mask_np = _bf16(np.triu(np.ones((P, P), np.float32)))

    in_maps = []
    meta = []
    for core in range(N_CORES):
        g, r = divmod(core, TPG)
        qs = slice(r * QC, (r + 1) * QC)
        fsl = slice(r * FFL, (r + 1) * FFL)
        v0 = r * VLOC
        vw = max(0, min(VLOC, V - v0))       # real vocab width of this shard
        wvoc = np.zeros((D, VLOC), np.float32)
        wvoc[:, :vw] = wteT_eff[:, v0:v0 + vw]
        vb_core = np.full((VLOC,), -1e4 if use_vb else 0.0, np.float32)
        vb_core[:vw] = vb[v0:v0 + vw]

        x = p['wte'][ids[g]] + p['wpe'][:T]          # [T, D]
        m = {
            'x0': _f32(_fm(x.T)),
            'wq': _bf16(np.stack([_fm(wq_l[l][:, qs]) for l in range(L)])),
            'wk': _bf16(np.stack([_fm(wk_l[l][:, qs]) for l in range(L)])),
            'wv': _bf16(np.stack([_fm(wv_l[l][:, qs]) for l in range(L)])),
            'bq': _f32(np.stack([_percol(bq_l[l][qs]) for l in range(L)])),
            'bk': _f32(np.stack([_percol(bk_l[l][qs]) for l in range(L)])),
            'bv': _bf16(np.stack([bv_l[l][qs][None, :] for l in range(L)])),
            'wp': _bf16(np.stack([_fm(wp_l[l][qs, :]) for l in range(L)])),
            'pb': _f32(np.stack([_percol(pb_l[l]) for l in range(L)])),
            'wfc': _bf16(np.stack([
                np.stack([_fm(wfc_l[l][:, fsl])[:, :, f * P:(f + 1) * P]
                          for f in range(FS)]) for l in range(L)])),
            'fcb': _f32(np.stack([_percol(fcb_l[l][fsl]) for l in range(L)])),
            'wfp': _bf16(np.stack([
                np.stack([_fm(wfp_l[l][fsl, :])[:, :, n * P:(n + 1) * P]
                          for n in range(DS)]) for l in range(L)])),
            'fpb': _f32(np.stack([_percol(fpb_l[l]) for l in range(L)])),
            'wvoc': _bf16(np.stack([_fm(wvoc)[:, :, vc * CH:(vc + 1) * CH]
                                    for vc in range(VT)])),
            'vb': _bf16(vb_core[None, :]),
            'mask': mask_np,
        }
        in_maps.append(m)
        meta.append({'g': g, 'r': r, 'v0': v0, 'vw': vw,
                     'padcount': 0 if use_vb else (VLOC - vw)})
    return in_maps, meta, use_vb


def run_on_device(input_ids, labels, params, trace=False):
    """Returns (logits [B,T,V] f32, loss f32, exec_time_ns or None)."""
    from concourse.bass_utils import run_bass_kernel_spmd

    in_maps, meta, use_vb = _prep_in_maps(input_ids, labels, params)
    key = ('prog', use_vb)
    if key not in _CACHE:
        _CACHE[key] = _build_program(use_vb)
    nc = _CACHE[key]

    res = run_bass_kernel_spmd(
        nc, in_maps, core_ids=list(range(N_CORES)), trace=trace)

    logits = np.empty((B, T, V), np.float32)
    S = np.zeros((B, T), np.float64)
    for core in range(N_CORES):
        md = meta[core]
        g, v0, vw = md['g'], md['v0'], md['vw']
        lg = res.results[core]['logits'].reshape(T, VLOC)
        logits[g, :, v0:v0 + vw] = lg[:, :vw]
        se = res.results[core]['sumexp']          # [P, TT]
        S[g] += se.T.reshape(T).astype(np.float64) - md['padcount']

    labels = np.asarray(labels)
    logz = np.log(S[:, :T - 1]).astype(np.float32)
    lab = labels[:, 1:]
    pick = np.take_along_axis(logits[:, :T - 1, :], lab[..., None], axis=-1)[..., 0]
    valid = lab != -100
    nll = np.where(valid, logz - pick, 0.0)
    loss = np.float32(nll.sum() / max(valid.sum(), 1))
    return logits, loss, res.exec_time_ns


def kernel(input_ids, labels, params):
    logits, loss, _ = run_on_device(input_ids, labels, params, trace=False)
    return logits, loss
